# revision 2
# baseline (speedup 1.0000x reference)
"""Trainium2 Bass kernel for nn_MultiHeadAttention (B=4, N=2048, E=768, H=8).

Sharding: 8 cores = 4 batches x 2 head-halves (4 heads each). Each core
computes QKV projections for its head slice, attention, and a partial output
projection; the host sums the two partials per batch and adds bo.

fp8 DoubleRow scheme (cost model: DR matmul = 0.5 cyc/out-row vs 1.0 bf16,
and each DR matmul contracts 2 chunks at once => 4x over bf16 per term):
  - QKV projections: x and weights quantized to fp8e4m3 (weights scaled x16)
    plus unscaled fp8 residuals. 3 accumulated DR terms x8@w8 + xr@w8 + x8@wr
    give ~bf16 accuracy at 2x the bf16 matmul throughput. V's bias enters via
    an augmented DR pair (ones-row x bias-row). Q/K biases via DVE.
  - scores: bf16 (the d=96 contraction cannot be DR-paired without a 2x
    projection cost), f32 PSUM.
  - softmax: ACT exp with scale=SC/256 (weights x16 twice), output fp8e4
    directly (Et8). Softmax denominator via ones-column of the V tile.
  - PV: DR pairs over key-chunks: Et8@(v8 + vr), 2 terms, f32 PSUM accum.
  - normalization: reciprocal of the ones-column row, DRAM-bounce partition
    broadcast, DVE multiply straight from PSUM -> bf16 c; Pool converts to
    u8/ur fp8 pair for the O projection.
  - O projection: DR pairs over heads, 3 terms u8@wo8 + ur@wo8 + u8@wor;
    result scaled by 1/256 at the bf16 output copy.
"""

import os
import sys

for _p in (
    "/root/.axon_site",
    "/root/.axon_site/_ro/trn_rl_repo",
    "/root/.axon_site/_ro/pypackages",
    "/opt/trn_rl_repo",
):
    if os.path.isdir(_p) and _p not in sys.path:
        sys.path.append(_p)

from contextlib import ExitStack

import ml_dtypes
import numpy as np

import concourse.bass as bass
import concourse.tile as tile
from concourse import mybir
from concourse.bass_utils import run_bass_kernel_spmd

BF16 = ml_dtypes.bfloat16
FP8 = ml_dtypes.float8_e4m3
E = 768
NT = 2048  # tokens
H = 8
D = 96
HC = 4  # heads per core
KC = 6  # 128-chunks over E
SC = 1.0 / float(np.sqrt(D))
SW = 16.0  # fp8 weight scale
QK_TERMS = 2  # 3 = clean (x8w8 + xrw8 + x8wr); 2 = x8w8 + x8wr

_NC_CACHE = {}

DR = mybir.MatmulPerfMode.DoubleRow


def _build_bass():
    f32 = mybir.dt.float32
    bf = mybir.dt.bfloat16
    f8 = mybir.dt.float8e4
    nc = bass.Bass(trn_type="TRN2", target_bir_lowering=False, debug=False)

    x8_d = nc.dram_tensor("x8", [128, KC, NT], f8, kind="ExternalInput").ap()
    xr_d = nc.dram_tensor("xr", [128, KC, NT], f8, kind="ExternalInput").ap()
    # packed weights, split by first-use so startup DMAs arrive in dep order
    wqp_d = nc.dram_tensor("wqp", [128, KC, 768], f8, kind="ExternalInput").ap()
    wkp_d = nc.dram_tensor("wkp", [128, KC, 768], f8, kind="ExternalInput").ap()
    # wv8: chunks 6/7 = bias + bias-residual rows
    wv8_d = nc.dram_tensor("wv8", [128, 8, 384], f8, kind="ExternalInput").ap()
    wvr_d = nc.dram_tensor("wvr", [128, KC, 384], f8, kind="ExternalInput").ap()
    wop_d = nc.dram_tensor("wop", [128, HC, 2 * E], f8, kind="ExternalInput").ap()
    bqk_d = nc.dram_tensor("bqk", [128, 2 * HC], f32, kind="ExternalInput").ap()
    yT_d = nc.dram_tensor("yT", [128, KC, NT], bf, kind="ExternalOutput").ap()

    Exp = mybir.ActivationFunctionType.Exp
    ESCALE = float(SC / (SW * SW))

    with tile.TileContext(nc) as tc, ExitStack() as ctx:
        consts = ctx.enter_context(tc.tile_pool(name="consts", bufs=1))
        big = ctx.enter_context(tc.tile_pool(name="big", bufs=1))

        wqp_sb = consts.tile([128, KC, 768], f8)
        wkp_sb = consts.tile([128, KC, 768], f8)
        wv8_sb = consts.tile([128, 8, 384], f8)
        wvr_sb = consts.tile([128, KC, 384], f8)
        wop_sb = consts.tile([128, HC, 2 * E], f8)
        bqk_sb = consts.tile([128, 2 * HC], f32)
        wq8_sb = wqp_sb[:, :, 0:384]
        wqr_sb = wqp_sb[:, :, 384:768]
        wk8_sb = wkp_sb[:, :, 0:384]
        wkr_sb = wkp_sb[:, :, 384:768]
        wo8_sb = wop_sb[:, :, 0:E]
        wor_sb = wop_sb[:, :, E : 2 * E]
        bq_sb = bqk_sb[:, 0:HC]
        bk_sb = bqk_sb[:, HC : 2 * HC]
        # aug lhsT for V bias: ones-rows pairing with bias + bias-residual rows
        augx = consts.tile([128, 2, 128], f8)
        zcol = consts.tile([128, 1], f32)  # exp bias (zeros)
        ones_row = consts.tile([1, 128], f32)  # lhsT for PE partition-broadcast
        nc.sync.dma_start(bqk_sb, bqk_d)

        # both aug slots are ones-rows: slot 0 pairs with the fp8 bias row,
        # slot 1 with the bias residual row (fp8^2-exact bias).
        nc.vector.memset(augx[:, :, :], 0.0)
        nc.vector.memset(augx[0:1, :, :], 1.0)
        nc.vector.memset(zcol, 0.0)
        nc.vector.memset(ones_row, 1.0)

        # DVE touches of DMA-loaded DVE operands (consume the DMA wait here).
        scratch = consts.tile([1, 8], f32)
        nc.vector.tensor_copy(scratch[0:1, 0:1], bq_sb[0:1, 0:1])
        nc.vector.tensor_copy(scratch[0:1, 1:2], bk_sb[0:1, 0:1])
        # ACT touch of zcol (used as the exp bias operand)
        scratch_a = consts.tile([1, 1], f32)
        nc.scalar.copy(scratch_a, zcol[0:1, :])

        qT_sb = big.tile([128, HC, NT], bf)  # [d(pad 128), head, token]
        kT_sb = big.tile([128, HC, NT], bf)
        # v, ones-augmented and zero-padded: [token%128, jc, head, 128]
        # cols 0:96 = v8, col 96 = 1.0 (softmax row-sum), 97:128 = 0
        v8_sb = big.tile([128, 16, HC, 128], f8)
        vr_sb = big.tile([128, 16, HC, 128], f8)
        out8_sb = big.tile([128, HC, NT], f8)  # [headdim(pad 128), head, token]
        outr_sb = big.tile([128, HC, NT], f8)
        scr_b = big.tile([1, 16], f32)  # per-block rb touch targets
        ypool = ctx.enter_context(tc.tile_pool(name="ypool", bufs=2))

        nc.vector.memset(v8_sb[:, :, :, 96:97], 1.0)
        nc.vector.memset(v8_sb[:, :, :, 97:128], 0.0)
        nc.vector.memset(vr_sb[:, :, :, 96:128], 0.0)

        with (
            tc.tile_pool(name="pss", bufs=2, space="PSUM") as pss,
            tc.tile_pool(name="psu", bufs=2, space="PSUM") as psu,
            tc.tile_pool(name="epool", bufs=8) as epool,
            tc.tile_pool(name="npool", bufs=2) as npool,
            tc.tile_pool(name="drp", bufs=1, space="DRAM") as drp,
        ):
            rdram = drp.tile([HC * 4, 512], f32)

            def attn_block(h, iq, pre_jp=None, pe_bcast=False):
                # one attention block = head h, query quarter iq (512 wide).
                bi = 4 * h + iq
                isl = slice(512 * iq, 512 * iq + 512)
                UT = psu.tile([128, 512], f32, tag="u")
                for jp in range(8):
                    if pre_jp is not None:
                        pre_jp(jp)
                    S = pss.tile([128, 2, 512], f32, tag="s")
                    for n in range(2):
                        jc = 2 * jp + n
                        nc.tensor.matmul(
                            S[:, n, :],
                            kT_sb[0:96, h, 128 * jc : 128 * jc + 128],
                            qT_sb[0:96, h, isl],
                            start=True,
                            stop=True,
                        )
                    Et = epool.tile([128, 2, 512], f8, tag="e")
                    nc.scalar.activation(Et, S, Exp, bias=zcol, scale=ESCALE)
                    nc.tensor.matmul(
                        UT,
                        v8_sb[:, 2 * jp : 2 * jp + 2, h, :],
                        Et,
                        start=(jp == 0),
                        stop=False,
                        perf_mode=DR,
                    )
                    nc.tensor.matmul(
                        UT,
                        vr_sb[:, 2 * jp : 2 * jp + 2, h, :],
                        Et,
                        start=False,
                        stop=(jp == 7),
                        perf_mode=DR,
                    )
                # normalization: c = UT * (1/r), r = row 96 (the ones column of
                # augmented V). Partition-broadcast of 1/r via DRAM bounce; the
                # last two blocks use a PE ones-matmul broadcast instead (the
                # bounce latency would land on the critical tail).
                rr = npool.tile([1, 512], f32, tag="rr")
                nc.vector.reciprocal(rr, UT[96:97, :])
                rb = npool.tile([128, 512], f32, tag="rb")
                if pe_bcast:
                    rbp = pss.tile([128, 2, 512], f32, tag="s")
                    nc.tensor.matmul(
                        rbp[:, 0, :], ones_row, rr, start=True, stop=True
                    )
                    nc.vector.tensor_copy(rb, rbp[:, 0, :])
                else:
                    nc.sync.dma_start(rdram[bi : bi + 1, :], rr)
                    row = rdram[bi : bi + 1, :]
                    rr_bcast = bass.AP(
                        tensor=row.tensor,
                        offset=row.offset,
                        ap=[[0, 128]] + [list(row.ap[-1])],
                    )
                    nc.sync.dma_start(rb, rr_bcast)
                    nc.vector.tensor_copy(scr_b[0:1, bi : bi + 1], rb[0:1, 0:1])
                cn = npool.tile([128, 512], bf, tag="c")
                nc.vector.tensor_mul(cn, UT, rb)
                # fp8 + residual for the O projection, on Pool
                nc.gpsimd.tensor_copy(out8_sb[:, h, isl], cn)
                nc.gpsimd.tensor_tensor(
                    outr_sb[:, h, isl], cn, out8_sb[:, h, isl],
                    mybir.AluOpType.subtract,
                )

            def phase3_quarter(q, psy, split_dma=False):
                isl = slice(512 * q, 512 * q + 512)
                y_sb = ypool.tile([128, KC, 512], bf, tag="ysb")
                for mc in range(KC):
                    py = psy.tile([128, 512], f32, tag="y")
                    msl = slice(128 * mc, 128 * mc + 128)
                    first = True
                    for p in range(2):
                        hsl = slice(2 * p, 2 * p + 2)
                        for wsb, osb in (
                            (wo8_sb, out8_sb),
                            (wo8_sb, outr_sb),
                            (wor_sb, out8_sb),
                        ):
                            nc.tensor.matmul(
                                py,
                                wsb[:, hsl, msl],
                                osb[:, hsl, isl],
                                start=first,
                                stop=(p == 1 and wsb is wor_sb),
                                perf_mode=DR,
                            )
                            first = False
                    nc.vector.tensor_scalar_mul(y_sb[:, mc, :], py, 1.0 / (SW * SW))
                    if split_dma and mc % 2 == 1:
                        nc.sync.dma_start(
                            yT_d[:, mc - 1 : mc + 1, isl],
                            y_sb[:, mc - 1 : mc + 1, :],
                        )
                if not split_dma:
                    nc.sync.dma_start(yT_d[:, :, isl], y_sb)

            with (
                tc.tile_pool(name="xpool", bufs=1) as xp,
                tc.tile_pool(name="ps1", bufs=2, space="PSUM") as ps1,
            ):
                x8_sb = xp.tile([128, KC, NT], f8)
                xr_sb = xp.tile([128, KC, NT], f8)
                # DMA transfers serialize; order by first-use so the head-0
                # K/Q projections and the first exps start ~8us in.
                nc.sync.dma_start(x8_sb[:, :, 0:512], x8_d[:, :, 0:512])
                nc.sync.dma_start(wkp_sb, wkp_d)
                nc.sync.dma_start(wqp_sb, wqp_d)
                nc.sync.dma_start(wv8_sb, wv8_d)
                nc.sync.dma_start(xr_sb[:, :, 0:512], xr_d[:, :, 0:512])
                nc.sync.dma_start(wvr_sb, wvr_d)
                nc.sync.dma_start(x8_sb[:, :, 512:NT], x8_d[:, :, 512:NT])
                nc.sync.dma_start(xr_sb[:, :, 512:NT], xr_d[:, :, 512:NT])
                nc.sync.dma_start(wop_sb, wop_d)

                if QK_TERMS == 3:
                    qk_pairs = [
                        (x8_sb, wq8_sb, wk8_sb),
                        (xr_sb, wq8_sb, wk8_sb),
                        (x8_sb, wqr_sb, wkr_sb),
                    ]
                else:
                    qk_pairs = [
                        (x8_sb, wq8_sb, wk8_sb),
                        (x8_sb, wqr_sb, wkr_sb),
                    ]
                n_qk = 3 * len(qk_pairs)

                def one_proj(h, i, wsel, dst, bias):
                    dsl = slice(96 * h, 96 * h + 96)
                    isl = slice(512 * i, 512 * i + 512)
                    pq = ps1.tile([128, 512], f32, tag="p1")
                    i_mm = 0
                    for pair in qk_pairs:
                        xsb, w = pair[0], pair[wsel]
                        for c in range(3):
                            csl = slice(2 * c, 2 * c + 2)
                            nc.tensor.matmul(
                                pq[0:96, :],
                                w[:, csl, dsl],
                                xsb[:, csl, isl],
                                start=(i_mm == 0),
                                stop=(i_mm == n_qk - 1),
                                perf_mode=DR,
                            )
                            i_mm += 1
                    nc.vector.tensor_scalar_add(
                        dst[0:96, h, isl], pq[0:96, :], bias[0:96, h : h + 1]
                    )

                def qk_proj(h):
                    # K first (attention needs every key quarter), Q after.
                    for i in range(4):
                        one_proj(h, i, 2, kT_sb, bk_sb)
                    for i in range(4):
                        one_proj(h, i, 1, qT_sb, bq_sb)

                def v_chunk(jc):
                    # term order tracks DMA arrival: x8+wv8, xr+wv8, x8+wvr
                    ksl = slice(128 * jc, 128 * jc + 128)
                    pv = ps1.tile([128, HC, D], f32, tag="p1")
                    mms = [(x8_sb[:, 2 * c : 2 * c + 2, ksl],
                            wv8_sb[:, 2 * c : 2 * c + 2, :]) for c in range(3)]
                    mms.append((augx[:, :, :], wv8_sb[:, 6:8, :]))
                    mms += [(xr_sb[:, 2 * c : 2 * c + 2, ksl],
                             wv8_sb[:, 2 * c : 2 * c + 2, :]) for c in range(3)]
                    mms += [(x8_sb[:, 2 * c : 2 * c + 2, ksl],
                             wvr_sb[:, 2 * c : 2 * c + 2, :]) for c in range(3)]
                    for i_mm, (lhs, rhs) in enumerate(mms):
                        nc.tensor.matmul(
                            pv, lhs, rhs,
                            start=(i_mm == 0),
                            stop=(i_mm == len(mms) - 1),
                            perf_mode=DR,
                        )
                    nc.vector.tensor_copy(v8_sb[:, jc, :, 0:96], pv)
                    nc.vector.scalar_tensor_tensor(
                        vr_sb[:, jc, :, 0:96],
                        pv,
                        1.0,
                        v8_sb[:, jc, :, 0:96],
                        mybir.AluOpType.mult,
                        mybir.AluOpType.subtract,
                    )

                # Pipelined prologue: head 0's K/Q projections and the V
                # projection are interleaved with the first attention block at
                # jp granularity, so ACT exp work starts as soon as the first
                # x quarter + weights arrive. S(jp) reads k-token-quarter
                # jp//2; PV(jp) reads v chunks 2jp,2jp+1 — each is emitted
                # just before the consumer.
                one_proj(0, 0, 2, kT_sb, bk_sb)
                one_proj(0, 0, 1, qT_sb, bq_sb)

                def pre(jp):
                    if jp > 0 and jp % 2 == 0:
                        one_proj(0, jp // 2, 2, kT_sb, bk_sb)
                    v_chunk(2 * jp)
                    v_chunk(2 * jp + 1)

                attn_block(0, 0, pre_jp=pre)
                one_proj(0, 1, 1, qT_sb, bq_sb)
                attn_block(0, 1)
                one_proj(0, 2, 1, qT_sb, bq_sb)
                one_proj(0, 3, 1, qT_sb, bq_sb)
                for h in range(1, HC):
                    qk_proj(h)
                    if h < HC - 1:
                        attn_block(h, 0)
                        attn_block(h, 1)

            # ps1/xpool closed: 2 PSUM banks free for early output projection.
            with tc.tile_pool(name="psy0", bufs=2, space="PSUM") as psy0:
                attn_block(HC - 1, 0)
                attn_block(HC - 1, 1)
                for h in range(HC):
                    attn_block(h, 2)
                # quarters 0/1 are complete; project them under the iq=2/3
                # attention blocks (emitted later = lower priority = fillers).
                phase3_quarter(0, psy0)
                attn_block(0, 3)
                attn_block(1, 3)
                phase3_quarter(1, psy0)
                attn_block(2, 3, pe_bcast=True)
                attn_block(3, 3, pe_bcast=True)
                phase3_quarter(2, psy0)

        # remaining PSUM free: last quarter fully pipelined
        with tc.tile_pool(name="psy1", bufs=4, space="PSUM") as psy1:
            phase3_quarter(3, psy1, split_dma=True)

    _split_multi_waits(nc)
    return nc


def _split_multi_waits(nc):
    """Walrus codegen allows only ONE sync wait on most compute-instruction
    structs. Hoist extra waits onto standalone EventSemaphore instructions
    inserted just before the offender on the same engine (semantically
    identical for in-order engines). DMA descriptors (queue-dispatched) are
    left alone."""
    import bass_rust

    n_split = 0
    for f in nc.m.functions:
        for blk in f.blocks:
            il = blk.instructions
            i = 0
            while i < len(il):
                inst = il[i]
                try:
                    si = inst.sync_info
                    waits = list(si.on_wait)
                except Exception:
                    i += 1
                    continue
                if len(waits) > 1 and inst.engine != mybir.EngineType.Unassigned:
                    for w in waits[:-1]:
                        ev = mybir.InstEventSemaphore(
                            name=f"wsplit_{n_split}", ins=[], outs=[]
                        )
                        n_split += 1
                        ev.engine = inst.engine
                        ev.sync_info = bass_rust.SyncInfo(on_wait=[w], on_update=[])
                        il.insert(i, ev)
                        i += 1
                    inst.sync_info = bass_rust.SyncInfo(
                        on_wait=[waits[-1]], on_update=list(si.on_update)
                    )
                i += 1
    return n_split


def _get_nc():
    if "nc" not in _NC_CACHE:
        _NC_CACHE["nc"] = _build_bass()
    return _NC_CACHE["nc"]


def _q8pair(a):
    """f32 array -> (fp8, unscaled fp8 residual)"""
    a8 = a.astype(FP8)
    ar = (a - a8.astype(np.float32)).astype(FP8)
    return a8, ar


def _to_lhsT(w):
    """[384, 768] weight (rows = output dims) -> [128, KC, 384] f32 lhsT chunks."""
    return np.ascontiguousarray(w.T.reshape(KC, 128, 384).transpose(1, 0, 2))


def _prep_half(Wq, bq, Wk, bk, Wv, bv, Wo, half):
    sl = slice(384 * half, 384 * (half + 1))
    wq8, wqr = _q8pair(_to_lhsT(Wq[sl, :].astype(np.float32) * SW))
    wk8, wkr = _q8pair(_to_lhsT(Wk[sl, :].astype(np.float32) * SW))
    wqp = np.concatenate([wq8, wqr], axis=2)
    wkp = np.concatenate([wk8, wkr], axis=2)

    wv8_6, wvr = _q8pair(_to_lhsT(Wv[sl, :].astype(np.float32) * SW))
    # wv8 chunks 6 (fp8 bias row) and 7 (bias residual row)
    wv8 = np.zeros((128, 8, 384), FP8)
    wv8[:, :KC] = wv8_6
    bv16 = np.asarray(bv)[sl].astype(np.float32) * SW
    b8 = bv16.astype(FP8)
    wv8[0, 6, :] = b8
    wv8[0, 7, :] = (bv16 - b8.astype(np.float32)).astype(FP8)

    WoT = Wo[:, sl].T.astype(np.float32) * SW  # [384, 768]
    wo_pad = np.zeros((HC, 128, E), np.float32)
    for h in range(HC):
        wo_pad[h, 0:96] = WoT[96 * h : 96 * h + 96]
    wo8, wor = _q8pair(np.ascontiguousarray(wo_pad.transpose(1, 0, 2)))
    wop = np.concatenate([wo8, wor], axis=2)

    bqk = np.zeros((128, 2 * HC), np.float32)
    for i, b in enumerate((bq, bk)):
        bb = np.asarray(b)[sl].astype(np.float32) * SW
        for h in range(HC):
            bqk[0:96, HC * i + h] = bb[96 * h : 96 * h + 96]

    return dict(wqp=wqp, wkp=wkp, wv8=wv8, wvr=wvr, wop=wop, bqk=bqk)


def _run(x, Wq, bq, Wk, bk, Wv, bv, Wo, bo, trace=False):
    x = np.asarray(x, dtype=np.float32)
    B = x.shape[0]
    halves = [
        _prep_half(np.asarray(Wq), np.asarray(bq), np.asarray(Wk), np.asarray(bk),
                   np.asarray(Wv), np.asarray(bv), np.asarray(Wo), hf)
        for hf in range(2)
    ]
    xTs = []
    for b in range(B):
        xT = np.ascontiguousarray(x[b].T.reshape(KC, 128, NT).transpose(1, 0, 2))
        xTs.append(_q8pair(xT))

    in_maps = []
    for c in range(8):
        b, hf = c // 2, c % 2
        m = dict(halves[hf])
        m["x8"], m["xr"] = xTs[b]
        in_maps.append(m)

    nc = _get_nc()
    res = run_bass_kernel_spmd(nc, in_maps, core_ids=list(range(8)), trace=trace)

    bo32 = np.asarray(bo, dtype=np.float32)
    y = np.empty((B, NT, E), np.float32)
    for b in range(B):
        p0 = res.results[2 * b]["yT"].astype(np.float32).transpose(1, 0, 2).reshape(E, NT)
        p1 = res.results[2 * b + 1]["yT"].astype(np.float32).transpose(1, 0, 2).reshape(E, NT)
        y[b] = (p0 + p1).T + bo32
    return y, res


def kernel(x, Wq, bq, Wk, bk, Wv, bv, Wo, bo):
    y, _ = _run(x, Wq, bq, Wk, bk, Wv, bv, Wo, bo, trace=False)
    return y
